# revision 9
# baseline (speedup 1.0000x reference)
"""Trainium2 Bass kernel for nn_BayesianKAN (3-layer B-spline KAN + KL).

Self-contained: takes FULL inputs (x, cm0, lv0, cm1, lv1, cm2, lv2),
shards batch across 8 NeuronCores (data-parallel), returns (out, kl).

Algorithm notes
---------------
The reference computes, per layer, a degree-3 clamped B-spline basis
expansion basis(x) in R^16 per (batch, feature) element followed by
einsum('bik,oik->bo', basis, cm), plus KL = 0.5*sum(exp(lv)+cm^2-1-lv).

Device-side reformulation:
 * The 16 clamped basis functions are an exact linear combination of 16
   translates U_n(x) = phi(13*clip(x) - n), n = -3..12, of the uniform
   cubic B-spline bump phi (support [0,4]).  The constant 16x16 mixing
   matrix M (|M|max = 6, cond ~38) is folded into the weights on the
   host: W'[n,i,o] = sum_j (M[n,j]/6) * cm[o,i,j].
 * 6*phi is evaluated elementwise as r^3 - 4*(r-1)+^3 with
   r = (2 - |w - c|)+, w = 13*clip(x), c = n+2 — two fused custom DVE
   ops per plane (exact piecewise identity, well conditioned).
 * The matmul runs on the PE in float32r (12 explicit mantissa bits,
   1 cycle/row at N>=512) using a 3-pass hi/lo compensation:
   B·W = Bh·Wh + Bl·Wh + Bh·Wl, with exact truncation splits.  This is
   fp32-accurate at 3 cycles/row (plain fp32 matmul costs 4).
 * Layout: features on partitions, batch on the free dim, so layer
   outputs land in PSUM already transposed for the next layer's basis
   computation.  The host pre-transposes x and post-transposes out.
"""

import operator

import numpy as np

import concourse.bacc as bacc
import concourse.mybir as mybir
import concourse.dve_ops as _dve_ops_mod
from concourse.dve_ops import DveOp
from concourse.dve_spec import (
    C0, C1, Spec, Src0, Src1, Zero, One, lower, maxx, minn, relu, sq,
    _has_src1,
)
from concourse.dve_uop import DveOpSpec
from concourse.tile import TileContext
from concourse.bass_utils import run_bass_kernel_spmd

# ---------------------------------------------------------------- constants
N_CORES = 8
BATCH = 8192
BSH = BATCH // N_CORES            # 1024 batch rows per core
SIZES = [256, 512, 512, 256]
NB = 16
DEGREE = 3
PASSES = 3                        # 3 = f32r hi/lo compensated, 1 = plain fp32
# per-layer compensation passes: L0/L1 full 3-pass, L2 single f32r pass
# (last-layer rounding is not amplified; measured end-to-end ~3.8e-4 rel)
LAYER_PASSES = (3, 3, 1)
NHALF = 512                       # psum bank width (matmul N)
CLIP_HI = np.float32(1.0 - 1e-6)
CBRT4 = float(np.cbrt(4.0))
# planes computed via the ACT-assisted path (rest pure-DVE); tunable balance
ACT_PLANES = set(range(2, 14))

F32 = mybir.dt.float32
F32R = mybir.dt.float32r
WDT = F32R if PASSES == 3 else F32


# ------------------------------------------------------- custom DVE ops
def _register_op(name, spec, subdim=False):
    for o in _dve_ops_mod.OPS:
        if o.name == name:
            return o
    shas = {}
    for ver in ("v3", "v4"):
        try:
            s = DveOpSpec(name=name, uops=lower(spec, ver=ver),
                          rd1_en=_has_src1(spec))
            shas[ver] = s.sha(ver)
        except Exception:
            pass
    op = DveOp(name, spec, subdim=subdim, uops_sha=shas)
    _dve_ops_mod.OPS.append(op)
    _dve_ops_mod.CUSTOM_DVE_SPECS[name] = spec
    _dve_ops_mod._SUB_OPCODE_FOR_NAME[name] = (
        _dve_ops_mod._CUSTOM_DVE_ROW_BASE + len(_dve_ops_mod.OPS) - 1
    )
    return op


_d = Src0 - C0
# r = relu(C1 - |w - C0|)
KAN_HAT_R = _register_op(
    "KAN_HAT_R",
    Spec(
        body=relu(C1 - maxx(_d, Zero - _d)),
        reference=lambda in0, in1, s0, s1, imm2: np.maximum(
            s1 - np.abs(in0 - s0), 0.0
        ).astype(np.float32),
    ),
)

_g = relu(Src0 - One) * C1
# U = r^3 - (C1*(r-1)+)^3 ; with C1 = cbrt(4) this is 6*phi
KAN_HAT_U = _register_op(
    "KAN_HAT_U",
    Spec(
        body=sq(Src0) * Src0 - sq(_g) * _g,
        reference=lambda in0, in1, s0, s1, imm2: (
            in0.astype(np.float32) ** 3
            - (np.float32(s1) * np.maximum(in0 - 1.0, 0.0).astype(np.float32)) ** 3
        ).astype(np.float32),
    ),
)

# w = C1 * clip(h, 0, C0)   (psum evacuation fused with clip+scale)
KAN_CLIP13 = _register_op(
    "KAN_CLIP13",
    Spec(
        body=minn(maxx(Src0, Zero), C0) * C1,
        reference=lambda in0, in1, s0, s1, imm2: (
            np.minimum(np.maximum(in0, 0.0), s0) * np.float32(s1)
        ).astype(np.float32),
    ),
)

# kl partial: out = cm^2 - lv ; accum_out = sum over free dim
KAN_KL = _register_op(
    "KAN_KL",
    Spec(
        body=sq(Src0) - Src1,
        accum=operator.add,
        reference=lambda in0, in1, s0, s1, imm2: (in0 * in0 - in1).astype(
            np.float32
        ),
    ),
)


# ------------------------------------------------------- host-side math
def _clamped_knots():
    n_interior = NB - DEGREE - 1
    interior = np.linspace(0.0, 1.0, n_interior + 2)[1:-1]
    return np.concatenate(
        [np.zeros(DEGREE + 1), interior, np.ones(DEGREE + 1)]
    )


def _bspline_basis64(x):
    """Cox-de Boor in float64, mirrors the reference. x: (...,) in [0,1]."""
    t = _clamped_knots()
    n_knots = NB + DEGREE + 1
    xe = np.clip(x, 0.0, 1.0 - 1e-6)[..., None]
    B = ((xe >= t[:-1]) & (xe < t[1:])).astype(np.float64)
    for p in range(1, DEGREE + 1):
        dl = t[p:n_knots - 1] - t[:n_knots - 1 - p]
        dr = t[p + 1:n_knots] - t[1:n_knots - p]
        left = np.where(dl > 0, (xe - t[:n_knots - 1 - p]) / np.where(dl > 0, dl, 1.0), 0.0)
        right = np.where(dr > 0, (t[p + 1:n_knots] - xe) / np.where(dr > 0, dr, 1.0), 0.0)
        B = left * B[..., :-1] + right * B[..., 1:]
    return B


def _u_feats64(x):
    """6*phi(13x - n) for n=-3..12. x: (...,) -> (..., 16)."""
    w = 13.0 * np.clip(x, 0.0, 1.0 - 1e-6)
    feats = []
    for n in range(-3, 13):
        s = np.abs(w - (n + 2.0))
        a = np.maximum(2.0 - s, 0.0)
        b = np.maximum(1.0 - s, 0.0)
        feats.append(a**3 - 4.0 * b**3)
    return np.stack(feats, axis=-1)


_M_CACHE = None


def _mix_matrix():
    """M/6 (16x16): basis16 = U16_unscaled @ (M/6)."""
    global _M_CACHE
    if _M_CACHE is None:
        xs = np.linspace(0.0, 1.0 - 1e-9, 20011)
        U = _u_feats64(xs)
        Bas = _bspline_basis64(xs)
        M, _, _, _ = np.linalg.lstsq(U, Bas, rcond=None)
        _M_CACHE = M  # already the /6-absorbed version (U unscaled by 1/6)
    return _M_CACHE


def _split12(a):
    """Exact hi/lo split: hi has <=11 explicit mantissa bits (f32r-safe)."""
    a = np.ascontiguousarray(a, np.float32)
    hi = (a.view(np.int32) & np.int32(~0xFFF)).view(np.float32)
    return hi, (a - hi)


def _prep_weights(cm):
    """cm: (O, F, 16) -> tiled W' layout [F/128, 16, 128, O] float32."""
    O, F, _ = cm.shape
    M = _mix_matrix()
    Wfull = np.einsum("oij,nj->nio", cm.astype(np.float64), M)
    W = Wfull.astype(np.float32)              # [16, F, O]
    W = W.reshape(NB, F // 128, 128, O).transpose(1, 0, 2, 3)
    return np.ascontiguousarray(W)            # [fc, 16, 128, O]


# ------------------------------------------------------- device kernel
_STATE = {}


def _build_nc():
    nc = bacc.Bacc(trn_type="TRN2", num_devices=N_CORES, debug=False)

    xw = nc.dram_tensor("xw", [SIZES[0], BSH], F32, kind="ExternalInput")
    wdecl = []
    for l in range(3):
        Fl, Ol = SIZES[l], SIZES[l + 1]
        shape = [Fl // 128, NB, 128, Ol]
        if LAYER_PASSES[l] == 3:
            wh = nc.dram_tensor(f"w{l}h", shape, WDT, kind="ExternalInput")
            wl = nc.dram_tensor(f"w{l}l", shape, WDT, kind="ExternalInput")
            wdecl.append((wh, wl))
        else:
            wdecl.append((nc.dram_tensor(f"w{l}h", shape, WDT, kind="ExternalInput"),))
    cmkl = nc.dram_tensor("cmkl", [128, 8192], F32, kind="ExternalInput")
    lvkl = nc.dram_tensor("lvkl", [128, 8192], F32, kind="ExternalInput")
    outT = nc.dram_tensor("outT", [SIZES[3], BSH], F32, kind="ExternalOutput")
    klp = nc.dram_tensor("klp", [128, 16], F32, kind="ExternalOutput")

    with TileContext(nc) as tc:
        with tc.tile_pool(name="xp", bufs=1) as xpool, \
             tc.tile_pool(name="pl", bufs=4) as plpool, \
             tc.tile_pool(name="tr", bufs=3) as trpool, \
             tc.tile_pool(name="wp", bufs=4) as wpool, \
             tc.tile_pool(name="kl", bufs=2) as klpool, \
             tc.tile_pool(name="ps", bufs=1, space="PSUM") as pspool:

            # ---- bias constants for ACT (const APs aren't pre-registered)
            bias_vals = sorted({-float(n - 1) for n in range(NB)} | {2.0})
            bias_tile = xpool.tile([128, len(bias_vals)], F32, name="biases", tag="biases")
            bias_ap = {}
            for bi, bv in enumerate(bias_vals):
                nc.vector.memset(bias_tile[:, bi:bi + 1], bv)
                bias_ap[bv] = bias_tile[:, bi:bi + 1]

            # hoist ACT_TABLE_LOAD: walrus inserts it before the first
            # ACTIVATE; issue a trivial one immediately so the ~1.3us load
            # overlaps the input DMA instead of stalling the first plane.
            warm_t = xpool.tile([128, 1], F32, name="actwarm", tag="actwarm")
            nc.scalar.activation(
                warm_t[:], bias_tile[:, 0:1],
                mybir.ActivationFunctionType.Abs,
                bias=bias_ap[2.0], scale=1.0,
            )

            # ---- persistent activation tiles (features on partitions)
            xt = {}
            for l in range(3):
                Fl = SIZES[l]
                xt[l] = [
                    xpool.tile([128, BSH], F32, name=f"x{l}_{i}", tag=f"x{l}_{i}")
                    for i in range(Fl // 128)
                ]
            outt = [
                xpool.tile([128, BSH], F32, name=f"out_{i}", tag=f"out_{i}")
                for i in range(SIZES[3] // 128)
            ]
            for i in range(SIZES[0] // 128):
                nc.sync.dma_start(
                    xt[0][i][:], xw.ap()[i * 128:(i + 1) * 128, :]
                )

            # ---- layers
            # L0 runs full-width (weights streamed once).  L1 and L2 run
            # per batch-half, pipelined: half 0's elementwise-bound L2
            # overlaps half 1's matmul-dense L1 (L1 weights stream twice).
            def emit_layer(l, bsl, nb_chunks):
                """Emit layer l for batch slice bsl split into nb_chunks
                psum column groups of width NHALF."""
                Fl, Ol = SIZES[l], SIZES[l + 1]
                nfc, noc = Fl // 128, Ol // 128
                last = l == 2
                lp = LAYER_PASSES[l]

                wid = bsl.stop - bsl.start
                # L2 reuses L0's bh=1 bank tags (free once L0 is evacuated),
                # keeping total distinct psum tags at 8 banks.
                def _pstag(oc, bh):
                    return f"ps_{oc}_{1}" if last else f"ps_{oc}_{bh}"
                ps = {
                    (oc, bh): pspool.tile([128, NHALF], F32,
                                          name=f"ps{l}_{oc}_{bh}",
                                          tag=_pstag(oc, bh))
                    for oc in range(noc) for bh in range(nb_chunks)
                }
                for fc in range(nfc):
                    for n in range(NB):
                        c = float(n - 3 + 2)  # center = n' + 2, n' = n-3
                        if n in ACT_PLANES:
                            s_t = trpool.tile([128, wid], F32, name="s_t", tag="s")
                            nc.scalar.activation(
                                s_t[:], xt[l][fc][:, bsl],
                                mybir.ActivationFunctionType.Abs,
                                bias=bias_ap[-c], scale=1.0,
                            )
                            r_t = trpool.tile([128, wid], F32, name="r_t", tag="r")
                            nc.scalar.activation(
                                r_t[:], s_t[:],
                                mybir.ActivationFunctionType.Relu,
                                bias=bias_ap[2.0], scale=-1.0,
                            )
                        else:
                            r_t = trpool.tile([128, wid], F32, name="r_t", tag="r")
                            nc.vector._custom_dve(
                                KAN_HAT_R, out=r_t[:], in0=xt[l][fc][:, bsl],
                                s0=c, s1=2.0,
                            )
                        b_t = trpool.tile(
                            [128, wid], F32 if lp == 3 else F32R,
                            name="b_t", tag="B",
                        )
                        nc.vector._custom_dve(
                            KAN_HAT_U, out=b_t[:], in0=r_t[:], s1=CBRT4,
                        )
                        if lp == 3:
                            bh_t = plpool.tile([128, wid], F32R, name="bh_t", tag="bh")
                            if n in ACT_PLANES:
                                nc.scalar.copy(bh_t[:], b_t[:])
                            else:
                                nc.vector.tensor_copy(bh_t[:], b_t[:])
                            bl_t = plpool.tile([128, wid], F32R, name="bl_t", tag="bl")
                            nc.vector.tensor_tensor(
                                bl_t[:], b_t[:], bh_t[:],
                                mybir.AluOpType.subtract,
                            )
                        wt_tiles = []
                        for wi, wd in enumerate(wdecl[l]):
                            wt = wpool.tile([128, Ol], WDT, name=f"wt{wi}", tag=f"w{wi}")
                            nc.sync.dma_start(wt[:], wd.ap()[fc, n])
                            wt_tiles.append(wt)
                        first = fc == 0 and n == 0
                        final = fc == nfc - 1 and n == NB - 1
                        for oc in range(noc):
                            osl = slice(oc * 128, (oc + 1) * 128)
                            for bh in range(nb_chunks):
                                pst = ps[(oc, bh)]
                                rsl = slice(bh * NHALF, (bh + 1) * NHALF)
                                if lp == 3:
                                    trip = (
                                        (wt_tiles[0], bh_t),
                                        (wt_tiles[0], bl_t),
                                        (wt_tiles[1], bh_t),
                                    )
                                else:
                                    trip = ((wt_tiles[0], b_t),)
                                for pi, (wt, rt) in enumerate(trip):
                                    nc.tensor.matmul(
                                        pst[:],
                                        wt[:, osl],
                                        rt[:, rsl],
                                        start=first and pi == 0,
                                        stop=final and pi == len(trip) - 1,
                                    )
                # evacuate psum
                for oc in range(noc):
                    for bh in range(nb_chunks):
                        pst = ps[(oc, bh)]
                        esl = slice(bsl.start + bh * NHALF,
                                    bsl.start + (bh + 1) * NHALF)
                        if last:
                            nc.scalar.copy(outt[oc][:, esl], pst[:])
                            nc.sync.dma_start(
                                outT.ap()[oc * 128:(oc + 1) * 128, esl],
                                outt[oc][:, esl],
                            )
                        else:
                            nc.vector._custom_dve(
                                KAN_CLIP13,
                                out=xt[l + 1][oc][:, esl],
                                in0=pst[:],
                                s0=float(CLIP_HI),
                                s1=13.0,
                            )

            emit_layer(0, slice(0, BSH), BSH // NHALF)
            for half in range(BSH // NHALF):
                hsl = slice(half * NHALF, (half + 1) * NHALF)
                emit_layer(1, hsl, 1)
                emit_layer(2, hsl, 1)

            # ---- KL partials
            klt = xpool.tile([128, 16], F32, name="klt", tag="klp")
            nchunk = 8
            cw = 8192 // nchunk
            for j in range(nchunk):
                csl = slice(j * cw, (j + 1) * cw)
                cm_t = klpool.tile([128, cw], F32, name="cm_t", tag="klcm")
                lv_t = klpool.tile([128, cw], F32, name="lv_t", tag="kllv")
                nc.sync.dma_start(cm_t[:], cmkl.ap()[:, csl])
                nc.sync.dma_start(lv_t[:], lvkl.ap()[:, csl])
                e_t = klpool.tile([128, cw], F32, name="e_t", tag="klsc")
                nc.scalar.activation(
                    e_t[:], lv_t[:], mybir.ActivationFunctionType.Exp,
                    accum_out=klt[:, j:j + 1],
                )
                s_t = klpool.tile([128, cw], F32, name="kls_t", tag="klsc")
                nc.vector._custom_dve(
                    KAN_KL, out=s_t[:], in0=cm_t[:], in1=lv_t[:],
                    accum_out=klt[:, nchunk + j:nchunk + j + 1],
                )
            nc.sync.dma_start(klp.ap(), klt[:])

    nc.finalize()
    return nc


def _get_nc():
    if "nc" not in _STATE:
        _STATE["nc"] = _build_nc()
    return _STATE["nc"]


def _prep_in_maps(x, cm0, lv0, cm1, lv1, cm2, lv2):
    x = np.ascontiguousarray(np.asarray(x, np.float32))
    cms = [np.asarray(c, np.float32) for c in (cm0, cm1, cm2)]
    lvs = [np.asarray(v, np.float32) for v in (lv0, lv1, lv2)]

    w = (np.float32(13.0) * np.clip(x, np.float32(0.0), CLIP_HI)).astype(
        np.float32
    )
    weights = {}
    for l in range(3):
        W = _prep_weights(cms[l])
        if LAYER_PASSES[l] == 3:
            hi, lo = _split12(W)
            weights[f"w{l}h"] = hi
            weights[f"w{l}l"] = lo
        else:
            weights[f"w{l}h"] = W

    CM = np.concatenate([c.ravel() for c in cms]).astype(np.float32)
    LV = np.concatenate([v.ravel() for v in lvs]).astype(np.float32)
    per = CM.size // N_CORES
    in_maps = []
    for c in range(N_CORES):
        m = dict(weights)
        m["xw"] = np.ascontiguousarray(
            w[c * BSH:(c + 1) * BSH, :].T
        )
        m["cmkl"] = CM[c * per:(c + 1) * per].reshape(128, -1)
        m["lvkl"] = LV[c * per:(c + 1) * per].reshape(128, -1)
        in_maps.append(m)
    return in_maps


def _run(in_maps, **kw):
    nc = _get_nc()
    return run_bass_kernel_spmd(nc, in_maps, core_ids=list(range(N_CORES)), **kw)


def _assemble(res):
    out = np.concatenate([r["outT"] for r in res.results], axis=1).T
    out = np.ascontiguousarray(out, np.float32)
    total = 0.0
    for r in res.results:
        total += float(r["klp"].astype(np.float64).sum())
    n_elem = sum(SIZES[i + 1] * SIZES[i] * NB for i in range(3))
    kl = np.float32(0.5 * (total - n_elem))
    return out, kl


def kernel(x, cm0, lv0, cm1, lv1, cm2, lv2):
    in_maps = _prep_in_maps(x, cm0, lv0, cm1, lv1, cm2, lv2)
    res = _run(in_maps)
    return _assemble(res)


# revision 11
# speedup vs baseline: 1.0518x; 1.0518x over previous
"""Trainium2 Bass kernel for nn_BayesianKAN (3-layer B-spline KAN + KL).

Self-contained: takes FULL inputs (x, cm0, lv0, cm1, lv1, cm2, lv2),
shards batch across 8 NeuronCores (data-parallel), returns (out, kl).

Algorithm notes
---------------
The reference computes, per layer, a degree-3 clamped B-spline basis
expansion basis(x) in R^16 per (batch, feature) element followed by
einsum('bik,oik->bo', basis, cm), plus KL = 0.5*sum(exp(lv)+cm^2-1-lv).

Device-side reformulation:
 * The 16 clamped basis functions are an exact linear combination of 16
   translates U_n(x) = phi(13*clip(x) - n), n = -3..12, of the uniform
   cubic B-spline bump phi (support [0,4]).  The constant 16x16 mixing
   matrix M (|M|max = 6, cond ~38) is folded into the weights on the
   host: W'[n,i,o] = sum_j (M[n,j]/6) * cm[o,i,j].
 * 6*phi is evaluated elementwise as r^3 - 4*(r-1)+^3 with
   r = (2 - |w - c|)+, w = 13*clip(x), c = n+2 — two fused custom DVE
   ops per plane (exact piecewise identity, well conditioned).
 * The matmul runs on the PE in float32r (12 explicit mantissa bits,
   1 cycle/row at N>=512) using a 3-pass hi/lo compensation:
   B·W = Bh·Wh + Bl·Wh + Bh·Wl, with exact truncation splits.  This is
   fp32-accurate at 3 cycles/row (plain fp32 matmul costs 4).
 * Layout: features on partitions, batch on the free dim, so layer
   outputs land in PSUM already transposed for the next layer's basis
   computation.  The host pre-transposes x and post-transposes out.
"""

import operator

import numpy as np

import concourse.bacc as bacc
import concourse.mybir as mybir
import concourse.dve_ops as _dve_ops_mod
from concourse.dve_ops import DveOp
from concourse.dve_spec import (
    C0, C1, Spec, Src0, Src1, Zero, One, lower, maxx, minn, relu, sq,
    _has_src1,
)
from concourse.dve_uop import DveOpSpec
from concourse.tile import TileContext
from concourse.bass_utils import run_bass_kernel_spmd

# ---------------------------------------------------------------- constants
N_CORES = 8
BATCH = 8192
BSH = BATCH // N_CORES            # 1024 batch rows per core
SIZES = [256, 512, 512, 256]
NB = 16
DEGREE = 3
PASSES = 3                        # 3 = f32r hi/lo compensated, 1 = plain fp32
# per-layer compensation passes: L0/L1 full 3-pass, L2 single f32r pass
# (last-layer rounding is not amplified; measured end-to-end ~3.8e-4 rel)
LAYER_PASSES = (3, 3, 1)
NHALF = 512                       # psum bank width (matmul N)
CLIP_HI = np.float32(1.0 - 1e-6)
CBRT4 = float(np.cbrt(4.0))
# planes computed via the ACT-assisted path (rest pure-DVE); tunable balance
ACT_PLANES = set(range(2, 14))

F32 = mybir.dt.float32
F32R = mybir.dt.float32r
WDT = F32R if PASSES == 3 else F32


# ------------------------------------------------------- custom DVE ops
def _register_op(name, spec, subdim=False):
    for o in _dve_ops_mod.OPS:
        if o.name == name:
            return o
    shas = {}
    for ver in ("v3", "v4"):
        try:
            s = DveOpSpec(name=name, uops=lower(spec, ver=ver),
                          rd1_en=_has_src1(spec))
            shas[ver] = s.sha(ver)
        except Exception:
            pass
    op = DveOp(name, spec, subdim=subdim, uops_sha=shas)
    _dve_ops_mod.OPS.append(op)
    _dve_ops_mod.CUSTOM_DVE_SPECS[name] = spec
    _dve_ops_mod._SUB_OPCODE_FOR_NAME[name] = (
        _dve_ops_mod._CUSTOM_DVE_ROW_BASE + len(_dve_ops_mod.OPS) - 1
    )
    return op


_d = Src0 - C0
# r = relu(C1 - |w - C0|)
KAN_HAT_R = _register_op(
    "KAN_HAT_R",
    Spec(
        body=relu(C1 - maxx(_d, Zero - _d)),
        reference=lambda in0, in1, s0, s1, imm2: np.maximum(
            s1 - np.abs(in0 - s0), 0.0
        ).astype(np.float32),
    ),
)

_g = relu(Src0 - One) * C1
# U = r^3 - (C1*(r-1)+)^3 ; with C1 = cbrt(4) this is 6*phi
KAN_HAT_U = _register_op(
    "KAN_HAT_U",
    Spec(
        body=sq(Src0) * Src0 - sq(_g) * _g,
        reference=lambda in0, in1, s0, s1, imm2: (
            in0.astype(np.float32) ** 3
            - (np.float32(s1) * np.maximum(in0 - 1.0, 0.0).astype(np.float32)) ** 3
        ).astype(np.float32),
    ),
)

# w = C1 * clip(h, 0, C0)   (psum evacuation fused with clip+scale)
KAN_CLIP13 = _register_op(
    "KAN_CLIP13",
    Spec(
        body=minn(maxx(Src0, Zero), C0) * C1,
        reference=lambda in0, in1, s0, s1, imm2: (
            np.minimum(np.maximum(in0, 0.0), s0) * np.float32(s1)
        ).astype(np.float32),
    ),
)

# kl partial: out = cm^2 - lv ; accum_out = sum over free dim
KAN_KL = _register_op(
    "KAN_KL",
    Spec(
        body=sq(Src0) - Src1,
        accum=operator.add,
        reference=lambda in0, in1, s0, s1, imm2: (in0 * in0 - in1).astype(
            np.float32
        ),
    ),
)


# ------------------------------------------------------- host-side math
def _clamped_knots():
    n_interior = NB - DEGREE - 1
    interior = np.linspace(0.0, 1.0, n_interior + 2)[1:-1]
    return np.concatenate(
        [np.zeros(DEGREE + 1), interior, np.ones(DEGREE + 1)]
    )


def _bspline_basis64(x):
    """Cox-de Boor in float64, mirrors the reference. x: (...,) in [0,1]."""
    t = _clamped_knots()
    n_knots = NB + DEGREE + 1
    xe = np.clip(x, 0.0, 1.0 - 1e-6)[..., None]
    B = ((xe >= t[:-1]) & (xe < t[1:])).astype(np.float64)
    for p in range(1, DEGREE + 1):
        dl = t[p:n_knots - 1] - t[:n_knots - 1 - p]
        dr = t[p + 1:n_knots] - t[1:n_knots - p]
        left = np.where(dl > 0, (xe - t[:n_knots - 1 - p]) / np.where(dl > 0, dl, 1.0), 0.0)
        right = np.where(dr > 0, (t[p + 1:n_knots] - xe) / np.where(dr > 0, dr, 1.0), 0.0)
        B = left * B[..., :-1] + right * B[..., 1:]
    return B


def _u_feats64(x):
    """6*phi(13x - n) for n=-3..12. x: (...,) -> (..., 16)."""
    w = 13.0 * np.clip(x, 0.0, 1.0 - 1e-6)
    feats = []
    for n in range(-3, 13):
        s = np.abs(w - (n + 2.0))
        a = np.maximum(2.0 - s, 0.0)
        b = np.maximum(1.0 - s, 0.0)
        feats.append(a**3 - 4.0 * b**3)
    return np.stack(feats, axis=-1)


_M_CACHE = None


def _mix_matrix():
    """M/6 (16x16): basis16 = U16_unscaled @ (M/6)."""
    global _M_CACHE
    if _M_CACHE is None:
        xs = np.linspace(0.0, 1.0 - 1e-9, 20011)
        U = _u_feats64(xs)
        Bas = _bspline_basis64(xs)
        M, _, _, _ = np.linalg.lstsq(U, Bas, rcond=None)
        _M_CACHE = M  # already the /6-absorbed version (U unscaled by 1/6)
    return _M_CACHE


def _split12(a):
    """Exact hi/lo split: hi has <=11 explicit mantissa bits (f32r-safe)."""
    a = np.ascontiguousarray(a, np.float32)
    hi = (a.view(np.int32) & np.int32(~0xFFF)).view(np.float32)
    return hi, (a - hi)


def _prep_weights(cm):
    """cm: (O, F, 16) -> tiled W' layout [F/128, 16, 128, O] float32."""
    O, F, _ = cm.shape
    M = _mix_matrix()
    Wfull = np.einsum("oij,nj->nio", cm.astype(np.float64), M)
    W = Wfull.astype(np.float32)              # [16, F, O]
    W = W.reshape(NB, F // 128, 128, O).transpose(1, 0, 2, 3)
    return np.ascontiguousarray(W)            # [fc, 16, 128, O]


# ------------------------------------------------------- device kernel
_STATE = {}


def _build_nc():
    nc = bacc.Bacc(trn_type="TRN2", num_devices=N_CORES, debug=False)

    xw = nc.dram_tensor("xw", [SIZES[0], BSH], F32, kind="ExternalInput")
    wdecl = []
    for l in range(3):
        Fl, Ol = SIZES[l], SIZES[l + 1]
        shape = [Fl // 128, NB, 128, Ol]
        if LAYER_PASSES[l] == 3:
            wh = nc.dram_tensor(f"w{l}h", shape, WDT, kind="ExternalInput")
            wl = nc.dram_tensor(f"w{l}l", shape, WDT, kind="ExternalInput")
            wdecl.append((wh, wl))
        else:
            wdecl.append((nc.dram_tensor(f"w{l}h", shape, WDT, kind="ExternalInput"),))
    cmkl = nc.dram_tensor("cmkl", [128, 8192], F32, kind="ExternalInput")
    lvkl = nc.dram_tensor("lvkl", [128, 8192], F32, kind="ExternalInput")
    outT = nc.dram_tensor("outT", [SIZES[3], BSH], F32, kind="ExternalOutput")
    klp = nc.dram_tensor("klp", [128, 16], F32, kind="ExternalOutput")

    with TileContext(nc) as tc:
        with tc.tile_pool(name="xp", bufs=1) as xpool, \
             tc.tile_pool(name="pl", bufs=4) as plpool, \
             tc.tile_pool(name="tr", bufs=3) as trpool, \
             tc.tile_pool(name="wp", bufs=4) as wpool, \
             tc.tile_pool(name="kl", bufs=2) as klpool, \
             tc.tile_pool(name="ps", bufs=1, space="PSUM") as pspool:

            # ---- bias constants for ACT (const APs aren't pre-registered)
            bias_vals = sorted({-float(n - 1) for n in range(NB)} | {2.0})
            bias_tile = xpool.tile([128, len(bias_vals)], F32, name="biases", tag="biases")
            bias_ap = {}
            for bi, bv in enumerate(bias_vals):
                nc.vector.memset(bias_tile[:, bi:bi + 1], bv)
                bias_ap[bv] = bias_tile[:, bi:bi + 1]

            # hoist ACT_TABLE_LOAD: walrus inserts it before the first
            # ACTIVATE; issue a trivial one immediately so the ~1.3us load
            # overlaps the input DMA instead of stalling the first plane.
            warm_t = xpool.tile([128, 1], F32, name="actwarm", tag="actwarm")
            nc.scalar.activation(
                warm_t[:], bias_tile[:, 0:1],
                mybir.ActivationFunctionType.Abs,
                bias=bias_ap[2.0], scale=1.0,
            )

            # ---- persistent activation tiles (features on partitions)
            xt = {}
            for l in range(3):
                Fl = SIZES[l]
                xt[l] = [
                    xpool.tile([128, BSH], F32, name=f"x{l}_{i}", tag=f"x{l}_{i}")
                    for i in range(Fl // 128)
                ]
            outt = [
                xpool.tile([128, BSH], F32, name=f"out_{i}", tag=f"out_{i}")
                for i in range(SIZES[3] // 128)
            ]
            for i in range(SIZES[0] // 128):
                nc.sync.dma_start(
                    xt[0][i][:], xw.ap()[i * 128:(i + 1) * 128, :]
                )

            # ---- layers
            # L0 runs full-width (weights streamed once).  L1 and L2 run
            # per batch-half, pipelined: half 0's elementwise-bound L2
            # overlaps half 1's matmul-dense L1 (L1 weights stream twice).
            def emit_layer(l, bsl, nb_chunks):
                """Emit layer l for batch slice bsl split into nb_chunks
                psum column groups of width NHALF."""
                Fl, Ol = SIZES[l], SIZES[l + 1]
                nfc, noc = Fl // 128, Ol // 128
                last = l == 2
                lp = LAYER_PASSES[l]

                wid = bsl.stop - bsl.start
                # L2 reuses L0's bh=1 bank tags (free once L0 is evacuated),
                # keeping total distinct psum tags at 8 banks.
                def _pstag(oc, bh):
                    # L2 (noc=2) maps onto L1's oc=2,3 banks (freed at evac)
                    return f"ps_{oc + 2}_{bh}" if last else f"ps_{oc}_{bh}"
                ps = {
                    (oc, bh): pspool.tile([128, NHALF], F32,
                                          name=f"ps{l}_{oc}_{bh}",
                                          tag=_pstag(oc, bh))
                    for oc in range(noc) for bh in range(nb_chunks)
                }
                for fc in range(nfc):
                    for n in range(NB):
                        c = float(n - 3 + 2)  # center = n' + 2, n' = n-3
                        if n in ACT_PLANES:
                            s_t = trpool.tile([128, wid], F32, name="s_t", tag="s")
                            nc.scalar.activation(
                                s_t[:], xt[l][fc][:, bsl],
                                mybir.ActivationFunctionType.Abs,
                                bias=bias_ap[-c], scale=1.0,
                            )
                            r_t = trpool.tile([128, wid], F32, name="r_t", tag="r")
                            nc.scalar.activation(
                                r_t[:], s_t[:],
                                mybir.ActivationFunctionType.Relu,
                                bias=bias_ap[2.0], scale=-1.0,
                            )
                        else:
                            r_t = trpool.tile([128, wid], F32, name="r_t", tag="r")
                            nc.vector._custom_dve(
                                KAN_HAT_R, out=r_t[:], in0=xt[l][fc][:, bsl],
                                s0=c, s1=2.0,
                            )
                        b_t = trpool.tile(
                            [128, wid], F32 if lp == 3 else F32R,
                            name="b_t", tag="B",
                        )
                        nc.vector._custom_dve(
                            KAN_HAT_U, out=b_t[:], in0=r_t[:], s1=CBRT4,
                        )
                        if lp == 3:
                            bh_t = plpool.tile([128, wid], F32R, name="bh_t", tag="bh")
                            if n in ACT_PLANES:
                                nc.scalar.copy(bh_t[:], b_t[:])
                            else:
                                nc.vector.tensor_copy(bh_t[:], b_t[:])
                            bl_t = plpool.tile([128, wid], F32R, name="bl_t", tag="bl")
                            nc.vector.tensor_tensor(
                                bl_t[:], b_t[:], bh_t[:],
                                mybir.AluOpType.subtract,
                            )
                        wt_tiles = []
                        for wi, wd in enumerate(wdecl[l]):
                            wt = wpool.tile([128, Ol], WDT, name=f"wt{wi}", tag=f"w{wi}")
                            nc.sync.dma_start(wt[:], wd.ap()[fc, n])
                            wt_tiles.append(wt)
                        first = fc == 0 and n == 0
                        final = fc == nfc - 1 and n == NB - 1
                        for oc in range(noc):
                            osl = slice(oc * 128, (oc + 1) * 128)
                            for bh in range(nb_chunks):
                                pst = ps[(oc, bh)]
                                rsl = slice(bh * NHALF, (bh + 1) * NHALF)
                                if lp == 3:
                                    trip = (
                                        (wt_tiles[0], bh_t),
                                        (wt_tiles[0], bl_t),
                                        (wt_tiles[1], bh_t),
                                    )
                                else:
                                    trip = ((wt_tiles[0], b_t),)
                                for pi, (wt, rt) in enumerate(trip):
                                    nc.tensor.matmul(
                                        pst[:],
                                        wt[:, osl],
                                        rt[:, rsl],
                                        start=first and pi == 0,
                                        stop=final and pi == len(trip) - 1,
                                    )
                # evacuate psum
                for oc in range(noc):
                    for bh in range(nb_chunks):
                        pst = ps[(oc, bh)]
                        esl = slice(bsl.start + bh * NHALF,
                                    bsl.start + (bh + 1) * NHALF)
                        if last:
                            nc.scalar.copy(outt[oc][:, esl], pst[:])
                            nc.sync.dma_start(
                                outT.ap()[oc * 128:(oc + 1) * 128, esl],
                                outt[oc][:, esl],
                            )
                        else:
                            nc.vector._custom_dve(
                                KAN_CLIP13,
                                out=xt[l + 1][oc][:, esl],
                                in0=pst[:],
                                s0=float(CLIP_HI),
                                s1=13.0,
                            )

            # ---- KL partials (emitted between L0 and L1 so the DVE/ACT
            # work lands where those engines have slack, not in the
            # elementwise-bound L2 tail)
            def emit_kl():
                klt = xpool.tile([128, 16], F32, name="klt", tag="klp")
                nchunk = 8
                cw = 8192 // nchunk
                for j in range(nchunk):
                    csl = slice(j * cw, (j + 1) * cw)
                    cm_t = klpool.tile([128, cw], F32, name="cm_t", tag="klcm")
                    lv_t = klpool.tile([128, cw], F32, name="lv_t", tag="kllv")
                    nc.sync.dma_start(cm_t[:], cmkl.ap()[:, csl])
                    nc.sync.dma_start(lv_t[:], lvkl.ap()[:, csl])
                    e_t = klpool.tile([128, cw], F32, name="e_t", tag="klsc")
                    nc.scalar.activation(
                        e_t[:], lv_t[:], mybir.ActivationFunctionType.Exp,
                        accum_out=klt[:, j:j + 1],
                    )
                    s_t = klpool.tile([128, cw], F32, name="kls_t", tag="klsc")
                    nc.vector._custom_dve(
                        KAN_KL, out=s_t[:], in0=cm_t[:], in1=lv_t[:],
                        accum_out=klt[:, nchunk + j:nchunk + j + 1],
                    )
                nc.sync.dma_start(klp.ap(), klt[:])

            emit_layer(0, slice(0, BSH), BSH // NHALF)
            emit_kl()
            emit_layer(1, slice(0, BSH), BSH // NHALF)
            emit_layer(2, slice(0, BSH), BSH // NHALF)

    nc.finalize()
    return nc


def _get_nc():
    if "nc" not in _STATE:
        _STATE["nc"] = _build_nc()
    return _STATE["nc"]


def _prep_in_maps(x, cm0, lv0, cm1, lv1, cm2, lv2):
    x = np.ascontiguousarray(np.asarray(x, np.float32))
    cms = [np.asarray(c, np.float32) for c in (cm0, cm1, cm2)]
    lvs = [np.asarray(v, np.float32) for v in (lv0, lv1, lv2)]

    w = (np.float32(13.0) * np.clip(x, np.float32(0.0), CLIP_HI)).astype(
        np.float32
    )
    weights = {}
    for l in range(3):
        W = _prep_weights(cms[l])
        if LAYER_PASSES[l] == 3:
            hi, lo = _split12(W)
            weights[f"w{l}h"] = hi
            weights[f"w{l}l"] = lo
        else:
            weights[f"w{l}h"] = W

    CM = np.concatenate([c.ravel() for c in cms]).astype(np.float32)
    LV = np.concatenate([v.ravel() for v in lvs]).astype(np.float32)
    per = CM.size // N_CORES
    in_maps = []
    for c in range(N_CORES):
        m = dict(weights)
        m["xw"] = np.ascontiguousarray(
            w[c * BSH:(c + 1) * BSH, :].T
        )
        m["cmkl"] = CM[c * per:(c + 1) * per].reshape(128, -1)
        m["lvkl"] = LV[c * per:(c + 1) * per].reshape(128, -1)
        in_maps.append(m)
    return in_maps


def _run(in_maps, **kw):
    nc = _get_nc()
    return run_bass_kernel_spmd(nc, in_maps, core_ids=list(range(N_CORES)), **kw)


def _assemble(res):
    out = np.concatenate([r["outT"] for r in res.results], axis=1).T
    out = np.ascontiguousarray(out, np.float32)
    total = 0.0
    for r in res.results:
        total += float(r["klp"].astype(np.float64).sum())
    n_elem = sum(SIZES[i + 1] * SIZES[i] * NB for i in range(3))
    kl = np.float32(0.5 * (total - n_elem))
    return out, kl


def kernel(x, cm0, lv0, cm1, lv1, cm2, lv2):
    in_maps = _prep_in_maps(x, cm0, lv0, cm1, lv1, cm2, lv2)
    res = _run(in_maps)
    return _assemble(res)


# revision 13
# speedup vs baseline: 1.2733x; 1.2106x over previous
"""Trainium2 Bass kernel for nn_BayesianKAN (3-layer B-spline KAN + KL).

Self-contained: takes FULL inputs (x, cm0, lv0, cm1, lv1, cm2, lv2),
shards batch across 8 NeuronCores (data-parallel), returns (out, kl).

Algorithm notes
---------------
The reference computes, per layer, a degree-3 clamped B-spline basis
expansion basis(x) in R^16 per (batch, feature) element followed by
einsum('bik,oik->bo', basis, cm), plus KL = 0.5*sum(exp(lv)+cm^2-1-lv).

Device-side reformulation:
 * The 16 clamped basis functions are an exact linear combination of 16
   translates U_n(x) = phi(13*clip(x) - n), n = -3..12, of the uniform
   cubic B-spline bump phi (support [0,4]).  The constant 16x16 mixing
   matrix M (|M|max = 6, cond ~38) is folded into the weights on the
   host: W'[n,i,o] = sum_j (M[n,j]/6) * cm[o,i,j].
 * 6*phi is evaluated elementwise as r^3 - 4*(r-1)+^3 with
   r = (2 - |w - c|)+, w = 13*clip(x), c = n+2 — two fused custom DVE
   ops per plane (exact piecewise identity, well conditioned).
 * The matmul runs on the PE in float32r (12 explicit mantissa bits,
   1 cycle/row at N>=512) using a 3-pass hi/lo compensation:
   B·W = Bh·Wh + Bl·Wh + Bh·Wl, with exact truncation splits.  This is
   fp32-accurate at 3 cycles/row (plain fp32 matmul costs 4).
 * Layout: features on partitions, batch on the free dim, so layer
   outputs land in PSUM already transposed for the next layer's basis
   computation.  The host pre-transposes x and post-transposes out.
"""

import operator

import numpy as np

import concourse.bacc as bacc
import concourse.mybir as mybir
import concourse.dve_ops as _dve_ops_mod
from concourse.dve_ops import DveOp
from concourse.dve_spec import (
    C0, C1, Spec, Src0, Src1, Zero, One, lower, maxx, minn, relu, sq,
    _has_src1,
)
from concourse.dve_uop import DveOpSpec
from concourse.tile import TileContext
from concourse.bass_utils import run_bass_kernel_spmd

# ---------------------------------------------------------------- constants
N_CORES = 8
BATCH = 8192
BSH = BATCH // N_CORES            # 1024 batch rows per core
SIZES = [256, 512, 512, 256]
NB = 16
DEGREE = 3
PASSES = 3                        # 3 = f32r hi/lo compensated, 1 = plain fp32
# per-layer matmul compensation mode:
#   3    = full 3-pass hi/lo (fp32-grade)
#   "2b" = weights exactly split (2 passes), basis RNE-rounded to f32r
#   1    = single f32r pass (both operands RNE-rounded by HW)
# (3,3,1) measured 1.58e-4 rel; (3,"2b",1) sim 6.6e-4; (3,1,1) sim 2.4e-3
LAYER_PASSES = (3, "2b", 1)
NHALF = 512                       # psum bank width (matmul N)
CLIP_HI = np.float32(1.0 - 1e-6)
CBRT4 = float(np.cbrt(4.0))
# planes computed via the ACT-assisted path (rest pure-DVE); tunable balance
ACT_PLANES = set(range(2, 14))

F32 = mybir.dt.float32
F32R = mybir.dt.float32r
WDT = F32R if PASSES == 3 else F32


# ------------------------------------------------------- custom DVE ops
def _register_op(name, spec, subdim=False):
    for o in _dve_ops_mod.OPS:
        if o.name == name:
            return o
    shas = {}
    for ver in ("v3", "v4"):
        try:
            s = DveOpSpec(name=name, uops=lower(spec, ver=ver),
                          rd1_en=_has_src1(spec))
            shas[ver] = s.sha(ver)
        except Exception:
            pass
    op = DveOp(name, spec, subdim=subdim, uops_sha=shas)
    _dve_ops_mod.OPS.append(op)
    _dve_ops_mod.CUSTOM_DVE_SPECS[name] = spec
    _dve_ops_mod._SUB_OPCODE_FOR_NAME[name] = (
        _dve_ops_mod._CUSTOM_DVE_ROW_BASE + len(_dve_ops_mod.OPS) - 1
    )
    return op


_d = Src0 - C0
# r = relu(C1 - |w - C0|)
KAN_HAT_R = _register_op(
    "KAN_HAT_R",
    Spec(
        body=relu(C1 - maxx(_d, Zero - _d)),
        reference=lambda in0, in1, s0, s1, imm2: np.maximum(
            s1 - np.abs(in0 - s0), 0.0
        ).astype(np.float32),
    ),
)

_g = relu(Src0 - One) * C1
# U = r^3 - (C1*(r-1)+)^3 ; with C1 = cbrt(4) this is 6*phi
KAN_HAT_U = _register_op(
    "KAN_HAT_U",
    Spec(
        body=sq(Src0) * Src0 - sq(_g) * _g,
        reference=lambda in0, in1, s0, s1, imm2: (
            in0.astype(np.float32) ** 3
            - (np.float32(s1) * np.maximum(in0 - 1.0, 0.0).astype(np.float32)) ** 3
        ).astype(np.float32),
    ),
)

# w = C1 * clip(h, 0, C0)   (psum evacuation fused with clip+scale)
KAN_CLIP13 = _register_op(
    "KAN_CLIP13",
    Spec(
        body=minn(maxx(Src0, Zero), C0) * C1,
        reference=lambda in0, in1, s0, s1, imm2: (
            np.minimum(np.maximum(in0, 0.0), s0) * np.float32(s1)
        ).astype(np.float32),
    ),
)

# kl partial: out = cm^2 - lv ; accum_out = sum over free dim
KAN_KL = _register_op(
    "KAN_KL",
    Spec(
        body=sq(Src0) - Src1,
        accum=operator.add,
        reference=lambda in0, in1, s0, s1, imm2: (in0 * in0 - in1).astype(
            np.float32
        ),
    ),
)


# ------------------------------------------------------- host-side math
def _clamped_knots():
    n_interior = NB - DEGREE - 1
    interior = np.linspace(0.0, 1.0, n_interior + 2)[1:-1]
    return np.concatenate(
        [np.zeros(DEGREE + 1), interior, np.ones(DEGREE + 1)]
    )


def _bspline_basis64(x):
    """Cox-de Boor in float64, mirrors the reference. x: (...,) in [0,1]."""
    t = _clamped_knots()
    n_knots = NB + DEGREE + 1
    xe = np.clip(x, 0.0, 1.0 - 1e-6)[..., None]
    B = ((xe >= t[:-1]) & (xe < t[1:])).astype(np.float64)
    for p in range(1, DEGREE + 1):
        dl = t[p:n_knots - 1] - t[:n_knots - 1 - p]
        dr = t[p + 1:n_knots] - t[1:n_knots - p]
        left = np.where(dl > 0, (xe - t[:n_knots - 1 - p]) / np.where(dl > 0, dl, 1.0), 0.0)
        right = np.where(dr > 0, (t[p + 1:n_knots] - xe) / np.where(dr > 0, dr, 1.0), 0.0)
        B = left * B[..., :-1] + right * B[..., 1:]
    return B


def _u_feats64(x):
    """6*phi(13x - n) for n=-3..12. x: (...,) -> (..., 16)."""
    w = 13.0 * np.clip(x, 0.0, 1.0 - 1e-6)
    feats = []
    for n in range(-3, 13):
        s = np.abs(w - (n + 2.0))
        a = np.maximum(2.0 - s, 0.0)
        b = np.maximum(1.0 - s, 0.0)
        feats.append(a**3 - 4.0 * b**3)
    return np.stack(feats, axis=-1)


_M_CACHE = None


def _mix_matrix():
    """M/6 (16x16): basis16 = U16_unscaled @ (M/6)."""
    global _M_CACHE
    if _M_CACHE is None:
        xs = np.linspace(0.0, 1.0 - 1e-9, 20011)
        U = _u_feats64(xs)
        Bas = _bspline_basis64(xs)
        M, _, _, _ = np.linalg.lstsq(U, Bas, rcond=None)
        _M_CACHE = M  # already the /6-absorbed version (U unscaled by 1/6)
    return _M_CACHE


def _split12(a):
    """Exact hi/lo split: hi has <=11 explicit mantissa bits (f32r-safe)."""
    a = np.ascontiguousarray(a, np.float32)
    hi = (a.view(np.int32) & np.int32(~0xFFF)).view(np.float32)
    return hi, (a - hi)


def _prep_weights(cm):
    """cm: (O, F, 16) -> tiled W' layout [F/128, 16, 128, O] float32."""
    O, F, _ = cm.shape
    M = _mix_matrix()
    Wfull = np.einsum("oij,nj->nio", cm.astype(np.float64), M)
    W = Wfull.astype(np.float32)              # [16, F, O]
    W = W.reshape(NB, F // 128, 128, O).transpose(1, 0, 2, 3)
    return np.ascontiguousarray(W)            # [fc, 16, 128, O]


# ------------------------------------------------------- device kernel
_STATE = {}


def _build_nc():
    nc = bacc.Bacc(trn_type="TRN2", num_devices=N_CORES, debug=False)

    xw = nc.dram_tensor("xw", [SIZES[0], BSH], F32, kind="ExternalInput")
    wdecl = []
    for l in range(3):
        Fl, Ol = SIZES[l], SIZES[l + 1]
        shape = [Fl // 128, NB, 128, Ol]
        if LAYER_PASSES[l] in (3, "2b"):
            wh = nc.dram_tensor(f"w{l}h", shape, WDT, kind="ExternalInput")
            wl = nc.dram_tensor(f"w{l}l", shape, WDT, kind="ExternalInput")
            wdecl.append((wh, wl))
        else:
            wdecl.append((nc.dram_tensor(f"w{l}h", shape, WDT, kind="ExternalInput"),))
    cmkl = nc.dram_tensor("cmkl", [128, 8192], F32, kind="ExternalInput")
    lvkl = nc.dram_tensor("lvkl", [128, 8192], F32, kind="ExternalInput")
    outT = nc.dram_tensor("outT", [SIZES[3], BSH], F32, kind="ExternalOutput")
    klp = nc.dram_tensor("klp", [128, 16], F32, kind="ExternalOutput")

    with TileContext(nc) as tc:
        with tc.tile_pool(name="xp", bufs=1) as xpool, \
             tc.tile_pool(name="pl", bufs=4) as plpool, \
             tc.tile_pool(name="tr", bufs=3) as trpool, \
             tc.tile_pool(name="wp", bufs=4) as wpool, \
             tc.tile_pool(name="kl", bufs=2) as klpool, \
             tc.tile_pool(name="ps", bufs=1, space="PSUM") as pspool:

            # ---- bias constants for ACT (const APs aren't pre-registered)
            bias_vals = sorted({-float(n - 1) for n in range(NB)} | {2.0})
            bias_tile = xpool.tile([128, len(bias_vals)], F32, name="biases", tag="biases")
            bias_ap = {}
            for bi, bv in enumerate(bias_vals):
                nc.vector.memset(bias_tile[:, bi:bi + 1], bv)
                bias_ap[bv] = bias_tile[:, bi:bi + 1]

            # hoist ACT_TABLE_LOAD: walrus inserts it before the first
            # ACTIVATE; issue a trivial one immediately so the ~1.3us load
            # overlaps the input DMA instead of stalling the first plane.
            warm_t = xpool.tile([128, 1], F32, name="actwarm", tag="actwarm")
            nc.scalar.activation(
                warm_t[:], bias_tile[:, 0:1],
                mybir.ActivationFunctionType.Abs,
                bias=bias_ap[2.0], scale=1.0,
            )

            # ---- persistent activation tiles (features on partitions)
            xt = {}
            for l in range(3):
                Fl = SIZES[l]
                xt[l] = [
                    xpool.tile([128, BSH], F32, name=f"x{l}_{i}", tag=f"x{l}_{i}")
                    for i in range(Fl // 128)
                ]
            outt = [
                xpool.tile([128, BSH], F32, name=f"out_{i}", tag=f"out_{i}")
                for i in range(SIZES[3] // 128)
            ]
            for i in range(SIZES[0] // 128):
                nc.sync.dma_start(
                    xt[0][i][:], xw.ap()[i * 128:(i + 1) * 128, :]
                )

            # ---- layers
            # L0 runs full-width (weights streamed once).  L1 and L2 run
            # per batch-half, pipelined: half 0's elementwise-bound L2
            # overlaps half 1's matmul-dense L1 (L1 weights stream twice).
            def emit_layer(l, bsl, nb_chunks):
                """Emit layer l for batch slice bsl split into nb_chunks
                psum column groups of width NHALF."""
                Fl, Ol = SIZES[l], SIZES[l + 1]
                nfc, noc = Fl // 128, Ol // 128
                last = l == 2
                lp = LAYER_PASSES[l]

                wid = bsl.stop - bsl.start
                # L2 reuses L0's bh=1 bank tags (free once L0 is evacuated),
                # keeping total distinct psum tags at 8 banks.
                def _pstag(oc, bh):
                    # L2 (noc=2) maps onto L1's oc=2,3 banks (freed at evac)
                    return f"ps_{oc + 2}_{bh}" if last else f"ps_{oc}_{bh}"
                ps = {
                    (oc, bh): pspool.tile([128, NHALF], F32,
                                          name=f"ps{l}_{oc}_{bh}",
                                          tag=_pstag(oc, bh))
                    for oc in range(noc) for bh in range(nb_chunks)
                }
                for fc in range(nfc):
                    for n in range(NB):
                        c = float(n - 3 + 2)  # center = n' + 2, n' = n-3
                        if n in ACT_PLANES:
                            s_t = trpool.tile([128, wid], F32, name="s_t", tag="s")
                            nc.scalar.activation(
                                s_t[:], xt[l][fc][:, bsl],
                                mybir.ActivationFunctionType.Abs,
                                bias=bias_ap[-c], scale=1.0,
                            )
                            r_t = trpool.tile([128, wid], F32, name="r_t", tag="r")
                            nc.scalar.activation(
                                r_t[:], s_t[:],
                                mybir.ActivationFunctionType.Relu,
                                bias=bias_ap[2.0], scale=-1.0,
                            )
                        else:
                            r_t = trpool.tile([128, wid], F32, name="r_t", tag="r")
                            nc.vector._custom_dve(
                                KAN_HAT_R, out=r_t[:], in0=xt[l][fc][:, bsl],
                                s0=c, s1=2.0,
                            )
                        b_t = trpool.tile(
                            [128, wid], F32 if lp == 3 else F32R,
                            name="b_t", tag="B",
                        )
                        nc.vector._custom_dve(
                            KAN_HAT_U, out=b_t[:], in0=r_t[:], s1=CBRT4,
                        )
                        if lp == 3:
                            bh_t = plpool.tile([128, wid], F32R, name="bh_t", tag="bh")
                            if n in ACT_PLANES:
                                nc.scalar.copy(bh_t[:], b_t[:])
                            else:
                                nc.vector.tensor_copy(bh_t[:], b_t[:])
                            bl_t = plpool.tile([128, wid], F32R, name="bl_t", tag="bl")
                            nc.vector.tensor_tensor(
                                bl_t[:], b_t[:], bh_t[:],
                                mybir.AluOpType.subtract,
                            )
                        wt_tiles = []
                        for wi, wd in enumerate(wdecl[l]):
                            wt = wpool.tile([128, Ol], WDT, name=f"wt{wi}", tag=f"w{wi}")
                            nc.sync.dma_start(wt[:], wd.ap()[fc, n])
                            wt_tiles.append(wt)
                        first = fc == 0 and n == 0
                        final = fc == nfc - 1 and n == NB - 1
                        for oc in range(noc):
                            osl = slice(oc * 128, (oc + 1) * 128)
                            for bh in range(nb_chunks):
                                pst = ps[(oc, bh)]
                                rsl = slice(bh * NHALF, (bh + 1) * NHALF)
                                if lp == 3:
                                    trip = (
                                        (wt_tiles[0], bh_t),
                                        (wt_tiles[0], bl_t),
                                        (wt_tiles[1], bh_t),
                                    )
                                elif lp == "2b":
                                    trip = (
                                        (wt_tiles[0], b_t),
                                        (wt_tiles[1], b_t),
                                    )
                                else:
                                    trip = ((wt_tiles[0], b_t),)
                                for pi, (wt, rt) in enumerate(trip):
                                    nc.tensor.matmul(
                                        pst[:],
                                        wt[:, osl],
                                        rt[:, rsl],
                                        start=first and pi == 0,
                                        stop=final and pi == len(trip) - 1,
                                    )
                # evacuate psum
                for oc in range(noc):
                    for bh in range(nb_chunks):
                        pst = ps[(oc, bh)]
                        esl = slice(bsl.start + bh * NHALF,
                                    bsl.start + (bh + 1) * NHALF)
                        if last:
                            nc.scalar.copy(outt[oc][:, esl], pst[:])
                            nc.sync.dma_start(
                                outT.ap()[oc * 128:(oc + 1) * 128, esl],
                                outt[oc][:, esl],
                            )
                        else:
                            nc.vector._custom_dve(
                                KAN_CLIP13,
                                out=xt[l + 1][oc][:, esl],
                                in0=pst[:],
                                s0=float(CLIP_HI),
                                s1=13.0,
                            )

            # ---- KL partials (emitted between L0 and L1 so the DVE/ACT
            # work lands where those engines have slack, not in the
            # elementwise-bound L2 tail)
            def emit_kl():
                klt = xpool.tile([128, 16], F32, name="klt", tag="klp")
                nchunk = 8
                cw = 8192 // nchunk
                for j in range(nchunk):
                    csl = slice(j * cw, (j + 1) * cw)
                    cm_t = klpool.tile([128, cw], F32, name="cm_t", tag="klcm")
                    lv_t = klpool.tile([128, cw], F32, name="lv_t", tag="kllv")
                    nc.sync.dma_start(cm_t[:], cmkl.ap()[:, csl])
                    nc.sync.dma_start(lv_t[:], lvkl.ap()[:, csl])
                    e_t = klpool.tile([128, cw], F32, name="e_t", tag="klsc")
                    nc.scalar.activation(
                        e_t[:], lv_t[:], mybir.ActivationFunctionType.Exp,
                        accum_out=klt[:, j:j + 1],
                    )
                    s_t = klpool.tile([128, cw], F32, name="kls_t", tag="klsc")
                    nc.vector._custom_dve(
                        KAN_KL, out=s_t[:], in0=cm_t[:], in1=lv_t[:],
                        accum_out=klt[:, nchunk + j:nchunk + j + 1],
                    )
                nc.sync.dma_start(klp.ap(), klt[:])

            emit_layer(0, slice(0, BSH), BSH // NHALF)
            emit_kl()
            emit_layer(1, slice(0, BSH), BSH // NHALF)
            emit_layer(2, slice(0, BSH), BSH // NHALF)

    nc.finalize()
    return nc


def _get_nc():
    if "nc" not in _STATE:
        _STATE["nc"] = _build_nc()
    return _STATE["nc"]


def _prep_in_maps(x, cm0, lv0, cm1, lv1, cm2, lv2):
    x = np.ascontiguousarray(np.asarray(x, np.float32))
    cms = [np.asarray(c, np.float32) for c in (cm0, cm1, cm2)]
    lvs = [np.asarray(v, np.float32) for v in (lv0, lv1, lv2)]

    w = (np.float32(13.0) * np.clip(x, np.float32(0.0), CLIP_HI)).astype(
        np.float32
    )
    weights = {}
    for l in range(3):
        W = _prep_weights(cms[l])
        if LAYER_PASSES[l] in (3, "2b"):
            hi, lo = _split12(W)
            weights[f"w{l}h"] = hi
            weights[f"w{l}l"] = lo
        else:
            weights[f"w{l}h"] = W

    CM = np.concatenate([c.ravel() for c in cms]).astype(np.float32)
    LV = np.concatenate([v.ravel() for v in lvs]).astype(np.float32)
    per = CM.size // N_CORES
    in_maps = []
    for c in range(N_CORES):
        m = dict(weights)
        m["xw"] = np.ascontiguousarray(
            w[c * BSH:(c + 1) * BSH, :].T
        )
        m["cmkl"] = CM[c * per:(c + 1) * per].reshape(128, -1)
        m["lvkl"] = LV[c * per:(c + 1) * per].reshape(128, -1)
        in_maps.append(m)
    return in_maps


def _run(in_maps, **kw):
    nc = _get_nc()
    return run_bass_kernel_spmd(nc, in_maps, core_ids=list(range(N_CORES)), **kw)


def _assemble(res):
    out = np.concatenate([r["outT"] for r in res.results], axis=1).T
    out = np.ascontiguousarray(out, np.float32)
    total = 0.0
    for r in res.results:
        total += float(r["klp"].astype(np.float64).sum())
    n_elem = sum(SIZES[i + 1] * SIZES[i] * NB for i in range(3))
    kl = np.float32(0.5 * (total - n_elem))
    return out, kl


def kernel(x, cm0, lv0, cm1, lv1, cm2, lv2):
    in_maps = _prep_in_maps(x, cm0, lv0, cm1, lv1, cm2, lv2)
    res = _run(in_maps)
    return _assemble(res)


# revision 14
# speedup vs baseline: 1.2739x; 1.0004x over previous
"""Trainium2 Bass kernel for nn_BayesianKAN (3-layer B-spline KAN + KL).

Self-contained: takes FULL inputs (x, cm0, lv0, cm1, lv1, cm2, lv2),
shards batch across 8 NeuronCores (data-parallel), returns (out, kl).

Algorithm notes
---------------
The reference computes, per layer, a degree-3 clamped B-spline basis
expansion basis(x) in R^16 per (batch, feature) element followed by
einsum('bik,oik->bo', basis, cm), plus KL = 0.5*sum(exp(lv)+cm^2-1-lv).

Device-side reformulation:
 * The 16 clamped basis functions are an exact linear combination of 16
   translates U_n(x) = phi(13*clip(x) - n), n = -3..12, of the uniform
   cubic B-spline bump phi (support [0,4]).  The constant 16x16 mixing
   matrix M (|M|max = 6, cond ~38) is folded into the weights on the
   host: W'[n,i,o] = sum_j (M[n,j]/6) * cm[o,i,j].
 * 6*phi is evaluated elementwise as r^3 - 4*(r-1)+^3 with
   r = (2 - |w - c|)+, w = 13*clip(x), c = n+2 — two fused custom DVE
   ops per plane (exact piecewise identity, well conditioned).
 * The matmul runs on the PE in float32r (12 explicit mantissa bits,
   1 cycle/row at N>=512) using a 3-pass hi/lo compensation:
   B·W = Bh·Wh + Bl·Wh + Bh·Wl, with exact truncation splits.  This is
   fp32-accurate at 3 cycles/row (plain fp32 matmul costs 4).
 * Layout: features on partitions, batch on the free dim, so layer
   outputs land in PSUM already transposed for the next layer's basis
   computation.  The host pre-transposes x and post-transposes out.
"""

import operator

import numpy as np

import concourse.bacc as bacc
import concourse.mybir as mybir
import concourse.dve_ops as _dve_ops_mod
from concourse.dve_ops import DveOp
from concourse.dve_spec import (
    C0, C1, Spec, Src0, Src1, Zero, One, lower, maxx, minn, relu, sq,
    _has_src1,
)
from concourse.dve_uop import DveOpSpec
from concourse.tile import TileContext
from concourse.bass_utils import run_bass_kernel_spmd

# ---------------------------------------------------------------- constants
N_CORES = 8
BATCH = 8192
BSH = BATCH // N_CORES            # 1024 batch rows per core
SIZES = [256, 512, 512, 256]
NB = 16
DEGREE = 3
PASSES = 3                        # 3 = f32r hi/lo compensated, 1 = plain fp32
# per-layer matmul compensation mode:
#   3    = full 3-pass hi/lo (fp32-grade)
#   "2b" = weights exactly split (2 passes), basis RNE-rounded to f32r
#   1    = single f32r pass (both operands RNE-rounded by HW)
# (3,3,1) measured 1.58e-4 rel; (3,"2b",1) sim 6.6e-4; (3,1,1) sim 2.4e-3
LAYER_PASSES = (3, "2b", 1)
NHALF = 512                       # psum bank width (matmul N)
CLIP_HI = np.float32(1.0 - 1e-6)
CBRT4 = float(np.cbrt(4.0))
# planes computed via the ACT-assisted path (rest pure-DVE); tunable balance
ACT_PLANES = set(range(2, 14))

F32 = mybir.dt.float32
F32R = mybir.dt.float32r
WDT = F32R if PASSES == 3 else F32


# ------------------------------------------------------- custom DVE ops
def _register_op(name, spec, subdim=False):
    for o in _dve_ops_mod.OPS:
        if o.name == name:
            return o
    shas = {}
    for ver in ("v3", "v4"):
        try:
            s = DveOpSpec(name=name, uops=lower(spec, ver=ver),
                          rd1_en=_has_src1(spec))
            shas[ver] = s.sha(ver)
        except Exception:
            pass
    op = DveOp(name, spec, subdim=subdim, uops_sha=shas)
    _dve_ops_mod.OPS.append(op)
    _dve_ops_mod.CUSTOM_DVE_SPECS[name] = spec
    _dve_ops_mod._SUB_OPCODE_FOR_NAME[name] = (
        _dve_ops_mod._CUSTOM_DVE_ROW_BASE + len(_dve_ops_mod.OPS) - 1
    )
    return op


_d = Src0 - C0
# r = relu(C1 - |w - C0|)
KAN_HAT_R = _register_op(
    "KAN_HAT_R",
    Spec(
        body=relu(C1 - maxx(_d, Zero - _d)),
        reference=lambda in0, in1, s0, s1, imm2: np.maximum(
            s1 - np.abs(in0 - s0), 0.0
        ).astype(np.float32),
    ),
)

_g = relu(Src0 - One) * C1
# U = r^3 - (C1*(r-1)+)^3 ; with C1 = cbrt(4) this is 6*phi
KAN_HAT_U = _register_op(
    "KAN_HAT_U",
    Spec(
        body=sq(Src0) * Src0 - sq(_g) * _g,
        reference=lambda in0, in1, s0, s1, imm2: (
            in0.astype(np.float32) ** 3
            - (np.float32(s1) * np.maximum(in0 - 1.0, 0.0).astype(np.float32)) ** 3
        ).astype(np.float32),
    ),
)

# w = C1 * clip(h, 0, C0)   (psum evacuation fused with clip+scale)
KAN_CLIP13 = _register_op(
    "KAN_CLIP13",
    Spec(
        body=minn(maxx(Src0, Zero), C0) * C1,
        reference=lambda in0, in1, s0, s1, imm2: (
            np.minimum(np.maximum(in0, 0.0), s0) * np.float32(s1)
        ).astype(np.float32),
    ),
)

# kl partial: out = cm^2 - lv ; accum_out = sum over free dim
KAN_KL = _register_op(
    "KAN_KL",
    Spec(
        body=sq(Src0) - Src1,
        accum=operator.add,
        reference=lambda in0, in1, s0, s1, imm2: (in0 * in0 - in1).astype(
            np.float32
        ),
    ),
)


# ------------------------------------------------------- host-side math
def _clamped_knots():
    n_interior = NB - DEGREE - 1
    interior = np.linspace(0.0, 1.0, n_interior + 2)[1:-1]
    return np.concatenate(
        [np.zeros(DEGREE + 1), interior, np.ones(DEGREE + 1)]
    )


def _bspline_basis64(x):
    """Cox-de Boor in float64, mirrors the reference. x: (...,) in [0,1]."""
    t = _clamped_knots()
    n_knots = NB + DEGREE + 1
    xe = np.clip(x, 0.0, 1.0 - 1e-6)[..., None]
    B = ((xe >= t[:-1]) & (xe < t[1:])).astype(np.float64)
    for p in range(1, DEGREE + 1):
        dl = t[p:n_knots - 1] - t[:n_knots - 1 - p]
        dr = t[p + 1:n_knots] - t[1:n_knots - p]
        left = np.where(dl > 0, (xe - t[:n_knots - 1 - p]) / np.where(dl > 0, dl, 1.0), 0.0)
        right = np.where(dr > 0, (t[p + 1:n_knots] - xe) / np.where(dr > 0, dr, 1.0), 0.0)
        B = left * B[..., :-1] + right * B[..., 1:]
    return B


def _u_feats64(x):
    """6*phi(13x - n) for n=-3..12. x: (...,) -> (..., 16)."""
    w = 13.0 * np.clip(x, 0.0, 1.0 - 1e-6)
    feats = []
    for n in range(-3, 13):
        s = np.abs(w - (n + 2.0))
        a = np.maximum(2.0 - s, 0.0)
        b = np.maximum(1.0 - s, 0.0)
        feats.append(a**3 - 4.0 * b**3)
    return np.stack(feats, axis=-1)


_M_CACHE = None


def _mix_matrix():
    """M/6 (16x16): basis16 = U16_unscaled @ (M/6)."""
    global _M_CACHE
    if _M_CACHE is None:
        xs = np.linspace(0.0, 1.0 - 1e-9, 20011)
        U = _u_feats64(xs)
        Bas = _bspline_basis64(xs)
        M, _, _, _ = np.linalg.lstsq(U, Bas, rcond=None)
        _M_CACHE = M  # already the /6-absorbed version (U unscaled by 1/6)
    return _M_CACHE


def _split12(a):
    """Exact hi/lo split: hi has <=11 explicit mantissa bits (f32r-safe)."""
    a = np.ascontiguousarray(a, np.float32)
    hi = (a.view(np.int32) & np.int32(~0xFFF)).view(np.float32)
    return hi, (a - hi)


def _prep_weights(cm):
    """cm: (O, F, 16) -> tiled W' layout [F/128, 16, 128, O] float32."""
    O, F, _ = cm.shape
    M = _mix_matrix()
    Wfull = np.einsum("oij,nj->nio", cm.astype(np.float64), M)
    W = Wfull.astype(np.float32)              # [16, F, O]
    W = W.reshape(NB, F // 128, 128, O).transpose(1, 0, 2, 3)
    return np.ascontiguousarray(W)            # [fc, 16, 128, O]


# ------------------------------------------------------- device kernel
_STATE = {}


def _build_nc():
    nc = bacc.Bacc(trn_type="TRN2", num_devices=N_CORES, debug=False)

    xw = nc.dram_tensor("xw", [SIZES[0], BSH], F32, kind="ExternalInput")
    wdecl = []
    for l in range(3):
        Fl, Ol = SIZES[l], SIZES[l + 1]
        shape = [Fl // 128, NB, 128, Ol]
        if LAYER_PASSES[l] in (3, "2b"):
            wh = nc.dram_tensor(f"w{l}h", shape, WDT, kind="ExternalInput")
            wl = nc.dram_tensor(f"w{l}l", shape, WDT, kind="ExternalInput")
            wdecl.append((wh, wl))
        else:
            wdecl.append((nc.dram_tensor(f"w{l}h", shape, WDT, kind="ExternalInput"),))
    cmkl = nc.dram_tensor("cmkl", [128, 8192], F32, kind="ExternalInput")
    lvkl = nc.dram_tensor("lvkl", [128, 8192], F32, kind="ExternalInput")
    outT = nc.dram_tensor("outT", [SIZES[3], BSH], F32, kind="ExternalOutput")
    klp = nc.dram_tensor("klp", [128, 16], F32, kind="ExternalOutput")

    with TileContext(nc) as tc:
        with tc.tile_pool(name="xp", bufs=1) as xpool, \
             tc.tile_pool(name="pl", bufs=4) as plpool, \
             tc.tile_pool(name="tr", bufs=3) as trpool, \
             tc.tile_pool(name="wp", bufs=4) as wpool, \
             tc.tile_pool(name="kl", bufs=2) as klpool, \
             tc.tile_pool(name="ps", bufs=1, space="PSUM") as pspool:

            # ---- bias constants for ACT (const APs aren't pre-registered)
            bias_vals = sorted({-float(n - 1) for n in range(NB)} | {2.0})
            bias_tile = xpool.tile([128, len(bias_vals)], F32, name="biases", tag="biases")
            bias_ap = {}
            for bi, bv in enumerate(bias_vals):
                nc.gpsimd.memset(bias_tile[:, bi:bi + 1], bv)
                bias_ap[bv] = bias_tile[:, bi:bi + 1]

            # hoist ACT_TABLE_LOAD: walrus inserts it before the first
            # ACTIVATE; issue a trivial one immediately so the ~1.3us load
            # overlaps the input DMA instead of stalling the first plane.
            warm_t = xpool.tile([128, 1], F32, name="actwarm", tag="actwarm")
            nc.scalar.activation(
                warm_t[:], bias_tile[:, 0:1],
                mybir.ActivationFunctionType.Abs,
                bias=bias_ap[2.0], scale=1.0,
            )

            # ---- persistent activation tiles (features on partitions)
            xt = {}
            for l in range(3):
                Fl = SIZES[l]
                xt[l] = [
                    xpool.tile([128, BSH], F32, name=f"x{l}_{i}", tag=f"x{l}_{i}")
                    for i in range(Fl // 128)
                ]
            outt = [
                xpool.tile([128, BSH], F32, name=f"out_{i}", tag=f"out_{i}")
                for i in range(SIZES[3] // 128)
            ]
            for i in range(SIZES[0] // 128):
                nc.sync.dma_start(
                    xt[0][i][:], xw.ap()[i * 128:(i + 1) * 128, :]
                )

            # ---- layers
            # L0 runs full-width (weights streamed once).  L1 and L2 run
            # per batch-half, pipelined: half 0's elementwise-bound L2
            # overlaps half 1's matmul-dense L1 (L1 weights stream twice).
            def emit_layer(l, bsl, nb_chunks):
                """Emit layer l for batch slice bsl split into nb_chunks
                psum column groups of width NHALF."""
                Fl, Ol = SIZES[l], SIZES[l + 1]
                nfc, noc = Fl // 128, Ol // 128
                last = l == 2
                lp = LAYER_PASSES[l]

                wid = bsl.stop - bsl.start
                # L2 reuses L0's bh=1 bank tags (free once L0 is evacuated),
                # keeping total distinct psum tags at 8 banks.
                def _pstag(oc, bh):
                    # L2 (noc=2) maps onto L1's oc=2,3 banks (freed at evac)
                    return f"ps_{oc + 2}_{bh}" if last else f"ps_{oc}_{bh}"
                ps = {
                    (oc, bh): pspool.tile([128, NHALF], F32,
                                          name=f"ps{l}_{oc}_{bh}",
                                          tag=_pstag(oc, bh))
                    for oc in range(noc) for bh in range(nb_chunks)
                }
                for fc in range(nfc):
                    for n in range(NB):
                        c = float(n - 3 + 2)  # center = n' + 2, n' = n-3
                        if n in ACT_PLANES:
                            s_t = trpool.tile([128, wid], F32, name="s_t", tag="s")
                            nc.scalar.activation(
                                s_t[:], xt[l][fc][:, bsl],
                                mybir.ActivationFunctionType.Abs,
                                bias=bias_ap[-c], scale=1.0,
                            )
                            r_t = trpool.tile([128, wid], F32, name="r_t", tag="r")
                            nc.scalar.activation(
                                r_t[:], s_t[:],
                                mybir.ActivationFunctionType.Relu,
                                bias=bias_ap[2.0], scale=-1.0,
                            )
                        else:
                            r_t = trpool.tile([128, wid], F32, name="r_t", tag="r")
                            nc.vector._custom_dve(
                                KAN_HAT_R, out=r_t[:], in0=xt[l][fc][:, bsl],
                                s0=c, s1=2.0,
                            )
                        b_t = trpool.tile(
                            [128, wid], F32 if lp == 3 else F32R,
                            name="b_t", tag="B",
                        )
                        nc.vector._custom_dve(
                            KAN_HAT_U, out=b_t[:], in0=r_t[:], s1=CBRT4,
                        )
                        if lp == 3:
                            bh_t = plpool.tile([128, wid], F32R, name="bh_t", tag="bh")
                            if n in ACT_PLANES:
                                nc.scalar.copy(bh_t[:], b_t[:])
                            else:
                                nc.vector.tensor_copy(bh_t[:], b_t[:])
                            bl_t = plpool.tile([128, wid], F32R, name="bl_t", tag="bl")
                            nc.vector.tensor_tensor(
                                bl_t[:], b_t[:], bh_t[:],
                                mybir.AluOpType.subtract,
                            )
                        wt_tiles = []
                        for wi, wd in enumerate(wdecl[l]):
                            wt = wpool.tile([128, Ol], WDT, name=f"wt{wi}", tag=f"w{wi}")
                            nc.sync.dma_start(wt[:], wd.ap()[fc, n])
                            wt_tiles.append(wt)
                        first = fc == 0 and n == 0
                        final = fc == nfc - 1 and n == NB - 1
                        for oc in range(noc):
                            osl = slice(oc * 128, (oc + 1) * 128)
                            for bh in range(nb_chunks):
                                pst = ps[(oc, bh)]
                                rsl = slice(bh * NHALF, (bh + 1) * NHALF)
                                if lp == 3:
                                    trip = (
                                        (wt_tiles[0], bh_t),
                                        (wt_tiles[0], bl_t),
                                        (wt_tiles[1], bh_t),
                                    )
                                elif lp == "2b":
                                    trip = (
                                        (wt_tiles[0], b_t),
                                        (wt_tiles[1], b_t),
                                    )
                                else:
                                    trip = ((wt_tiles[0], b_t),)
                                for pi, (wt, rt) in enumerate(trip):
                                    nc.tensor.matmul(
                                        pst[:],
                                        wt[:, osl],
                                        rt[:, rsl],
                                        start=first and pi == 0,
                                        stop=final and pi == len(trip) - 1,
                                    )
                # evacuate psum
                for oc in range(noc):
                    for bh in range(nb_chunks):
                        pst = ps[(oc, bh)]
                        esl = slice(bsl.start + bh * NHALF,
                                    bsl.start + (bh + 1) * NHALF)
                        if last:
                            nc.scalar.copy(outt[oc][:, esl], pst[:])
                            nc.sync.dma_start(
                                outT.ap()[oc * 128:(oc + 1) * 128, esl],
                                outt[oc][:, esl],
                            )
                        else:
                            nc.vector._custom_dve(
                                KAN_CLIP13,
                                out=xt[l + 1][oc][:, esl],
                                in0=pst[:],
                                s0=float(CLIP_HI),
                                s1=13.0,
                            )

            # ---- KL partials (emitted between L0 and L1 so the DVE/ACT
            # work lands where those engines have slack, not in the
            # elementwise-bound L2 tail)
            def emit_kl():
                klt = xpool.tile([128, 16], F32, name="klt", tag="klp")
                nchunk = 8
                cw = 8192 // nchunk
                for j in range(nchunk):
                    csl = slice(j * cw, (j + 1) * cw)
                    cm_t = klpool.tile([128, cw], F32, name="cm_t", tag="klcm")
                    lv_t = klpool.tile([128, cw], F32, name="lv_t", tag="kllv")
                    nc.sync.dma_start(cm_t[:], cmkl.ap()[:, csl])
                    nc.sync.dma_start(lv_t[:], lvkl.ap()[:, csl])
                    e_t = klpool.tile([128, cw], F32, name="e_t", tag="klsc")
                    nc.scalar.activation(
                        e_t[:], lv_t[:], mybir.ActivationFunctionType.Exp,
                        accum_out=klt[:, j:j + 1],
                    )
                    s_t = klpool.tile([128, cw], F32, name="kls_t", tag="klsc")
                    nc.vector._custom_dve(
                        KAN_KL, out=s_t[:], in0=cm_t[:], in1=lv_t[:],
                        accum_out=klt[:, nchunk + j:nchunk + j + 1],
                    )
                nc.sync.dma_start(klp.ap(), klt[:])

            emit_layer(0, slice(0, BSH), BSH // NHALF)
            emit_kl()
            emit_layer(1, slice(0, BSH), BSH // NHALF)
            emit_layer(2, slice(0, BSH), BSH // NHALF)

    nc.finalize()
    return nc


def _get_nc():
    if "nc" not in _STATE:
        _STATE["nc"] = _build_nc()
    return _STATE["nc"]


def _prep_in_maps(x, cm0, lv0, cm1, lv1, cm2, lv2):
    x = np.ascontiguousarray(np.asarray(x, np.float32))
    cms = [np.asarray(c, np.float32) for c in (cm0, cm1, cm2)]
    lvs = [np.asarray(v, np.float32) for v in (lv0, lv1, lv2)]

    w = (np.float32(13.0) * np.clip(x, np.float32(0.0), CLIP_HI)).astype(
        np.float32
    )
    weights = {}
    for l in range(3):
        W = _prep_weights(cms[l])
        if LAYER_PASSES[l] in (3, "2b"):
            hi, lo = _split12(W)
            weights[f"w{l}h"] = hi
            weights[f"w{l}l"] = lo
        else:
            weights[f"w{l}h"] = W

    CM = np.concatenate([c.ravel() for c in cms]).astype(np.float32)
    LV = np.concatenate([v.ravel() for v in lvs]).astype(np.float32)
    per = CM.size // N_CORES
    in_maps = []
    for c in range(N_CORES):
        m = dict(weights)
        m["xw"] = np.ascontiguousarray(
            w[c * BSH:(c + 1) * BSH, :].T
        )
        m["cmkl"] = CM[c * per:(c + 1) * per].reshape(128, -1)
        m["lvkl"] = LV[c * per:(c + 1) * per].reshape(128, -1)
        in_maps.append(m)
    return in_maps


def _run(in_maps, **kw):
    nc = _get_nc()
    return run_bass_kernel_spmd(nc, in_maps, core_ids=list(range(N_CORES)), **kw)


def _assemble(res):
    out = np.concatenate([r["outT"] for r in res.results], axis=1).T
    out = np.ascontiguousarray(out, np.float32)
    total = 0.0
    for r in res.results:
        total += float(r["klp"].astype(np.float64).sum())
    n_elem = sum(SIZES[i + 1] * SIZES[i] * NB for i in range(3))
    kl = np.float32(0.5 * (total - n_elem))
    return out, kl


def kernel(x, cm0, lv0, cm1, lv1, cm2, lv2):
    in_maps = _prep_in_maps(x, cm0, lv0, cm1, lv1, cm2, lv2)
    res = _run(in_maps)
    return _assemble(res)
